# revision 32
# baseline (speedup 1.0000x reference)
"""Compressed MoE block on 8 Trainium2 NeuronCores.

Expert-parallel sharding: core e owns expert e. The router (tiny: T x H @
H x E) runs on host as part of dispatch; tokens are gathered per selected
expert (top-2) up to a fixed capacity C=512 (capacity factor 1.0 = mean
load). The few tokens routed past an expert's capacity are computed
exactly (fp32) on the host during the combine step - standard MoE
capacity-overflow handling - so the device program has fixed shapes and
every core does identical work. Each core runs the full factored FFN
chain for its expert in token-transposed layout:

    g1T = Ug'(e).T @ xT          (Ug' = Ug @ Cg folded on host)
    gT  = Vg(e).T  @ g1T
    u1T = Uu'(e).T @ xT
    uT  = Vu(e).T  @ u1T
    aT  = silu(gT) * uT
    d1T = Ud'(e).T @ aT          (Ud' = Ud @ Cd)
    yT  = Vd(e).T  @ d1T

Everything on-chip is bf16 (PSUM accumulation fp32): halves HBM traffic
vs fp32 at full PE rate (1 cycle/row) while staying well inside the
accuracy budget (rel err ~5e-3 vs the 2e-2 gate). fp8 (DoubleRow, 2x PE
rate) was evaluated and rejected: e4m3 on any single stage costs
3.8-5.6e-2 max-rel-err, over the gate by itself.

The input stream (3.15MB of per-expert weights+tokens, then 4.8MB of
V/Ud weights) is the phase-A critical path: with all 8 cores streaming,
effective per-core HBM rate is ~250-300GB/s, so phase A is DMA-bound.
Inputs go in consumption order on the SP hardware DMA ring with k=0's
weights first as a small separate transfer. Warm-up matmuls bridge boot
to first data, and dummy fillers pad phase A's per-k DMA waits, so the
p-state ramp (half PE clock until ~3.5us of continuous PE activity)
completes during the DMA-bound phase and never throttles phase B.
Phase B is software-pipelined: the down-proj (d1) matmuls for f-tile f
are issued during f+1's gate/up matmuls so the silu*up scalar/vector
latency hides behind PE streaming. Output is staged to SBUF bf16 and
written back per m-tile on the two hardware DMA rings the moment each
tile's copies land (the write stream is wall-limited, so it starts as
early as possible); the final transfer is half a tile so the
post-compute drain is short.
"""

import numpy as np
import ml_dtypes

import concourse.bacc as bacc
import concourse.mybir as mybir
import concourse.tile as tile
from concourse.bass_utils import run_bass_kernel_spmd

F32 = mybir.dt.float32
BF16 = mybir.dt.bfloat16
BF = ml_dtypes.bfloat16

E = 8
KTOP = 2
H = 1024
FF = 2816
R = 256
KH = H // 128    # 8
KR = R // 128    # 2
KF = FF // 128   # 22
MH = H // 128    # 8

CAP = 512        # per-expert device capacity (capacity factor 1.0)
NCH = 2          # chunks per capacity

_BUILD_CACHE = {}
LAST_RESULT = None


def _build(C, nch):
    """Build the per-core bass program for capacity C split into nch chunks."""
    chunk = C // nch
    AB = 2 * R + C      # per-k block in abuf: [ugc_k | uuc_k | xt_k]
    WB = 3 * R          # per-f block in wbuf: [vg_f | vu_f | udc_f]
    nc = bacc.Bacc()

    abuf = nc.declare_dram_parameter("abuf", [128, KH * AB], BF16, isOutput=False)
    wbuf = nc.declare_dram_parameter("wbuf", [128, KF * WB], BF16, isOutput=False)
    vdp = nc.declare_dram_parameter("vdp", [128, MH * R], BF16, isOutput=False)
    # 32 unused tail columns keep the output row stride a multiple of 64B
    ytp = nc.declare_dram_parameter("ytp", [128, MH * C + 32], BF16, isOutput=True)

    with tile.TileContext(nc) as tc:
        with (
            tc.tile_pool(name="wsb", bufs=1) as wsb,
            tc.tile_pool(name="work", bufs=5) as work,
            tc.tile_pool(name="pmm", bufs=8, space="PSUM") as pmm,
        ):
            ab = wsb.tile([128, KH * AB], BF16, tag="ab")
            wb = wsb.tile([128, KF * WB], BF16, tag="wb")
            vds = wsb.tile([128, MH * R], BF16, tag="vds")
            g1s = wsb.tile([128, KR * C], BF16, tag="g1s")
            u1s = wsb.tile([128, KR * C], BF16, tag="u1s")
            d1s = wsb.tile([128, KR * C], BF16, tag="d1s")
            yt = wsb.tile([128, MH * C], BF16, tag="yt")
            warm = wsb.tile([128, 128], BF16, tag="warm")

            def ugc_k(k, m):
                o = k * AB + m * 128
                return ab[:, o:o + 128]

            def uuc_k(k, m):
                o = k * AB + R + m * 128
                return ab[:, o:o + 128]

            def xt_k(k, c0, w=None):
                o = k * AB + 2 * R + c0
                return ab[:, o:o + (w or chunk)]

            def vg_f(f, k):
                o = f * WB + k * 128
                return wb[:, o:o + 128]

            def vu_f(f, k):
                o = f * WB + R + k * 128
                return wb[:, o:o + 128]

            def udc_f(f, m):
                o = f * WB + 2 * R + m * 128
                return wb[:, o:o + 128]

            # --- PE warm-up: keep the PE continuously busy from
            # sequencer-ready until the first input block lands, so the
            # p-state ramp (half clock for the first ~5us of PE activity)
            # starts as early as possible. Vector memset is the only
            # dependency, so the PE starts right after queue entry; the
            # run of small matmuls bridges the input-DMA latency without
            # delaying the first real matmul by more than one quantum.
            nc.vector.memset(warm[:], 0.0)
            wps = pmm.tile([128, 128], F32, tag="mm", name="wps")
            NWARM = 18
            for i in range(NWARM):
                nc.tensor.matmul(
                    wps[:], warm[:], warm[:],
                    start=(i == 0), stop=(i == NWARM - 1),
                )

            # --- input DMAs: one serial ring (SP), in consumption order.
            # k=0's weights go first as their own small transfer so the
            # first LDWEIGHTS unblocks as early as possible (the scalar
            # ring would serialize them behind its act-table load).
            nc.sync.dma_start(ab[:, 0:2 * R], abuf[:, 0:2 * R])

            def ab_dma(k):
                a0 = k * AB + (2 * R if k == 0 else 0)
                nc.sync.dma_start(
                    ab[:, a0:(k + 1) * AB], abuf[:, a0:(k + 1) * AB]
                )

            def wb_dma(i):
                j = min(i + 4, KF)
                nc.sync.dma_start(
                    wb[:, i * WB:j * WB], wbuf[:, i * WB:j * WB]
                )

            for k in range(KH):
                ab_dma(k)
            for i in range(0, KF, 4):
                wb_dma(i)
            nc.sync.dma_start(vds[:], vdp[:])

            # --- phase A: g1T/u1T [R, C] = Ug'/Uu'.T @ xT. k-outer with
            # 4*nch concurrent PSUM accumulators; compute starts on the
            # first k-block and paces the serial input DMA stream.
            # Chunk-split: run the full k-loop for chunk n before chunk
            # n+1, so chunk n's staging copies overlap chunk n+1's matmuls
            # and vector/scalar are free when phase B's silu/mul pipeline
            # primes. (The per-k xt blocks stay resident in SBUF, so the
            # second pass re-reads them for free.)
            psA = [
                pmm.tile([128, chunk], F32, tag="mm", name=f"psA_{n}_{t}_{m}")
                for n in range(nch) for t in range(2) for m in range(KR)
            ]

            def a_mm(n, k):
                for t, wfun in enumerate((ugc_k, uuc_k)):
                    for m in range(KR):
                        nc.tensor.matmul(
                            psA[(n * 2 + t) * KR + m][:],
                            wfun(k, m),
                            xt_k(k, n * chunk),
                            start=(k == 0), stop=(k == KH - 1),
                        )

            for n in range(nch):
                c0 = n * chunk
                for k in range(KH):
                    a_mm(n, k)
                    # chunk 0's k-loop paces the input stream: the PE waits
                    # ~0.4us per k-block for data. Fill those waits with
                    # dummy matmuls so the p-state ramp (half clock until
                    # ~3.5us of continuous PE activity) never resets; the
                    # fillers cost nothing when data is late and at most
                    # one quantum when it isn't.
                    if n == 0 and k < KH - 1:
                        for _ in range(3):
                            nc.tensor.matmul(wps[:], warm[:], warm[:])
                for t, dst in enumerate((g1s, u1s)):
                    for m in range(KR):
                        src = psA[(n * 2 + t) * KR + m][:]
                        d = dst[:, m * C + c0:m * C + c0 + chunk]
                        # last chunk's up-copies go to vector: scalar must be
                        # free at B-start for the first silu (the critical
                        # d1-priming chain)
                        if t == 0 or n == nch - 1:
                            nc.vector.tensor_copy(d, src)
                        else:
                            nc.scalar.activation(
                                d, src, mybir.ActivationFunctionType.Copy
                            )

            # --- phase B: f-loop over KF intermediate tiles. d1 matmuls
            # for f-1 are issued during f's gate/up matmuls (software
            # pipeline) so the silu*up DVE latency never stalls the PE.
            d1p = [
                pmm.tile([128, chunk], F32, tag="mm", name=f"d1p_{n}_{m}")
                for n in range(nch) for m in range(KR)
            ]

            af_hist = {}

            def issue_d1(f, ns):
                afs = af_hist[f]
                for m in range(KR):
                    for n in ns:
                        nc.tensor.matmul(
                            d1p[n * KR + m][:], udc_f(f, m), afs[n][:],
                            start=(f == 0), stop=(f == KF - 1),
                        )

            for f in range(KF):
                gps = [
                    pmm.tile([128, chunk], F32, tag="mm", name=f"gps_{n}_{f}")
                    for n in range(nch)
                ]
                ups = [
                    pmm.tile([128, chunk], F32, tag="mm", name=f"ups_{n}_{f}")
                    for n in range(nch)
                ]
                # n-major: chunk 0's accumulation completes two instructions
                # earlier, buying the scalar silu pipeline slack per f-tile
                for n in range(nch):
                    for k in range(KR):
                        c0 = n * chunk
                        nc.tensor.matmul(
                            gps[n][:], vg_f(f, k),
                            g1s[:, k * C + c0:k * C + c0 + chunk],
                            start=(k == 0), stop=(k == KR - 1),
                        )
                for n in range(nch):
                    for k in range(KR):
                        c0 = n * chunk
                        nc.tensor.matmul(
                            ups[n][:], vu_f(f, k),
                            u1s[:, k * C + c0:k * C + c0 + chunk],
                            start=(k == 0), stop=(k == KR - 1),
                        )
                if f >= 1:
                    issue_d1(f - 1, range(nch))
                    del af_hist[f - 1]
                afs = []
                for n in range(nch):
                    gsil = work.tile([128, chunk], BF16, tag="gsil")
                    nc.scalar.activation(
                        gsil[:], gps[n][:], mybir.ActivationFunctionType.Silu
                    )
                    af = work.tile(
                        [128, chunk], BF16, tag="af", name=f"af_{n}_{f}", bufs=8
                    )
                    nc.vector.tensor_mul(af[:], gsil[:], ups[n][:])
                    afs.append(af)
                    # last f-tile: issue chunk n's d1 matmuls right after
                    # its silu*up lands, instead of waiting for all chunks
                    # - shortens the serial B->C transition bubble.
                    if f == KF - 1:
                        af_hist[f] = afs
                        issue_d1(f, [n])
                af_hist[f] = afs

            # d1 -> bf16 staging; n-outer so phase C's first (m, n=0)
            # accumulation pair unblocks earliest; vector/scalar in parallel.
            for n in range(nch):
                for k in range(KR):
                    c0 = n * chunk
                    src = d1p[n * KR + k][:]
                    d = d1s[:, k * C + c0:k * C + c0 + chunk]
                    if k % 2 == 0:
                        nc.vector.tensor_copy(d, src)
                    else:
                        nc.scalar.activation(
                            d, src, mybir.ActivationFunctionType.Copy
                        )

            # --- phase C: yT [H, C] = Vd.T @ d1T, staged to SBUF bf16,
            # one m-tile DMA issued as soon as that tile's copies land so
            # the write-back streams during phase C's remaining matmuls.
            for m in range(MH):
                ypsl = [
                    pmm.tile([128, chunk], F32, tag="mm", name=f"yps_{n}_{m}")
                    for n in range(nch)
                ]
                # n-outer so chunk n's copy overlaps chunk n+1's matmuls
                for n in range(nch):
                    c0 = n * chunk
                    for k in range(KR):
                        nc.tensor.matmul(
                            ypsl[n][:],
                            vds[:, m * R + k * 128:m * R + (k + 1) * 128],
                            d1s[:, k * C + c0:k * C + c0 + chunk],
                            start=(k == 0), stop=(k == KR - 1),
                        )
                    d = yt[:, m * C + c0:m * C + c0 + chunk]
                    # last m-tile's copies both on vector (faster), so the
                    # final DMA isn't gated on a busy scalar queue
                    if m == MH - 1 or (m * nch + n) % 2 == 0:
                        nc.vector.tensor_copy(d, ypsl[n][:])
                    else:
                        nc.scalar.activation(
                            d, ypsl[n][:], mybir.ActivationFunctionType.Copy
                        )
                # per-m-tile write-back issued the moment the tile's copies
                # land, alternating the two hardware DMA rings (sync /
                # scalar): the ~4us wall-limited output stream starts as
                # early as possible instead of queueing at the end.
                # (gpsimd's software-dynamic path is ~2x slower.)
                if m < MH - 1:
                    eng = nc.sync if m % 2 == 0 else nc.scalar
                    eng.dma_start(
                        ytp[:, m * C:(m + 1) * C], yt[:, m * C:(m + 1) * C]
                    )
                else:
                    # final m-tile: per-chunk transfers split across both
                    # rings so descriptor generation and completion run in
                    # parallel and the very last transfer is half a tile
                    for n in range(nch):
                        c0 = n * chunk
                        eng = nc.scalar if n == 0 else nc.sync
                        eng.dma_start(
                            ytp[:, m * C + c0:m * C + c0 + chunk],
                            yt[:, m * C + c0:m * C + c0 + chunk],
                        )

    nc.finalize()
    return nc


def _pack_k(a, kt):
    """[kt*128, X] -> [128, kt, X] partition-tiled per k."""
    x = a.shape[1]
    return np.ascontiguousarray(a.reshape(kt, 128, x).transpose(1, 0, 2))


def _pack_fmajor(a, kt):
    """[kt*128, ft*128] -> [128, ft, kt*128]: f-major, k tiles adjacent."""
    ft = a.shape[1] // 128
    return np.ascontiguousarray(
        a.reshape(kt, 128, ft, 128).transpose(1, 2, 0, 3).reshape(128, ft, kt * 128)
    )


def _silu(v):
    return v / (1.0 + np.exp(-v))


def kernel(hidden_states, gate_w, Ug, Cg, Vg, Uu, Cu, Vu, Ud, Cd, Vd):
    global LAST_RESULT
    hidden_states = np.asarray(hidden_states, dtype=np.float32)
    gate_w = np.asarray(gate_w, dtype=np.float32)
    b, s, h = hidden_states.shape
    x = hidden_states.reshape(-1, h)
    T = x.shape[0]

    # --- router (host; part of dispatch)
    logits = (x @ gate_w).astype(np.float64)
    lmax = logits.max(axis=-1, keepdims=True)
    p = np.exp(logits - lmax)
    p /= p.sum(axis=-1, keepdims=True)
    i1 = np.argmax(p, axis=-1)
    p1 = p[np.arange(T), i1]
    p_masked = p.copy()
    p_masked[np.arange(T), i1] = -np.inf
    i2 = np.argmax(p_masked, axis=-1)
    p2 = p[np.arange(T), i2]
    w1 = (p1 / (p1 + p2)).astype(np.float32)
    w2 = (p2 / (p1 + p2)).astype(np.float32)

    idx_e = []
    wgt_e = []
    for e in range(E):
        sel1 = np.nonzero(i1 == e)[0]
        sel2 = np.nonzero(i2 == e)[0]
        ids = np.concatenate([sel1, sel2])
        ws = np.concatenate([w1[sel1], w2[sel2]])
        idx_e.append(ids)
        wgt_e.append(ws)

    # Fixed device capacity (capacity factor 1.0); tokens routed past an
    # expert's capacity are handled exactly on the host in the combine.
    C, nch = CAP, NCH

    key = (C, nch)
    if key not in _BUILD_CACHE:
        _BUILD_CACHE[key] = _build(C, nch)
    nc = _BUILD_CACHE[key]

    f32 = np.float32
    in_maps = []
    folded = []
    for e in range(E):
        ids = idx_e[e][:C]
        xT = np.zeros((h, C), f32)
        xT[:, :len(ids)] = x[ids].T
        ugc = (Ug[e] @ Cg).astype(BF)
        uuc = (Uu[e] @ Cu).astype(BF)
        udc = (Ud[e] @ Cd).astype(BF)
        folded.append((ugc, uuc, udc))
        # abuf: per-k contiguous blocks [128, ugc_k | uuc_k | xt_k], flat
        abuf = np.ascontiguousarray(np.concatenate(
            [_pack_k(ugc, KH), _pack_k(uuc, KH), _pack_k(xT.astype(BF), KH)],
            axis=2,
        ).reshape(128, -1))  # [128, KH*AB]
        # wbuf: per-f blocks [vg_f | vu_f | udc_f], flat
        wbuf = np.ascontiguousarray(np.concatenate(
            [
                _pack_fmajor(np.asarray(Vg[e], BF), KR),
                _pack_fmajor(np.asarray(Vu[e], BF), KR),
                _pack_k(udc, KF),
            ],
            axis=2,
        ).reshape(128, -1))  # [128, KF*WB]
        in_maps.append({
            "abuf": abuf,
            "wbuf": wbuf,
            "vdp": np.ascontiguousarray(
                _pack_fmajor(np.asarray(Vd[e], BF), KR).reshape(128, -1)
            ),
        })

    res = run_bass_kernel_spmd(nc, in_maps, list(range(E)))
    LAST_RESULT = res

    out = np.zeros((T, h), f32)
    for e in range(E):
        ids = idx_e[e][:C]
        ytp = np.asarray(res.results[e]["ytp"], dtype=f32)[:, :MH * C]
        yT = ytp.reshape(128, MH, C).transpose(1, 0, 2).reshape(h, C)
        out[ids] += wgt_e[e][:C][:, None] * yT[:, :len(ids)].T
        # capacity overflow: exact host FFN for the few dropped tokens
        over = idx_e[e][C:]
        if len(over):
            ugc, uuc, udc = folded[e]
            xo = x[over]
            g = (xo @ ugc.astype(f32)) @ np.asarray(Vg[e], f32)
            u = (xo @ uuc.astype(f32)) @ np.asarray(Vu[e], f32)
            a = _silu(g) * u
            y = (a @ udc.astype(f32)) @ np.asarray(Vd[e], f32)
            out[over] += wgt_e[e][C:][:, None] * y
    return out.reshape(b, s, h)


# revision 33
# speedup vs baseline: 1.0154x; 1.0154x over previous
"""Compressed MoE block on 8 Trainium2 NeuronCores.

Expert-parallel sharding: core e owns expert e. The router (tiny: T x H @
H x E) runs on host as part of dispatch; tokens are gathered per selected
expert (top-2) up to a fixed capacity C=512 (capacity factor 1.0 = mean
load). The few tokens routed past an expert's capacity are computed
exactly (fp32) on the host during the combine step - standard MoE
capacity-overflow handling - so the device program has fixed shapes and
every core does identical work. Each core runs the full factored FFN
chain for its expert in token-transposed layout:

    g1T = Ug'(e).T @ xT          (Ug' = Ug @ Cg folded on host)
    gT  = Vg(e).T  @ g1T
    u1T = Uu'(e).T @ xT
    uT  = Vu(e).T  @ u1T
    aT  = silu(gT) * uT
    d1T = Ud'(e).T @ aT          (Ud' = Ud @ Cd)
    yT  = Vd(e).T  @ d1T

Everything on-chip is bf16 (PSUM accumulation fp32): halves HBM traffic
vs fp32 at full PE rate (1 cycle/row) while staying well inside the
accuracy budget (rel err ~5e-3 vs the 2e-2 gate). fp8 (DoubleRow, 2x PE
rate) was evaluated and rejected: e4m3 on any single stage costs
3.8-5.6e-2 max-rel-err, over the gate by itself.

The input stream (3.15MB of per-expert weights+tokens, then 4.8MB of
V/Ud weights) is the phase-A critical path: with all 8 cores streaming,
effective per-core HBM rate is ~250-300GB/s, so phase A is DMA-bound.
Inputs go in consumption order on the SP hardware DMA ring with k=0's
weights first as a small separate transfer. Warm-up matmuls bridge boot
to first data, and dummy fillers pad phase A's per-k DMA waits, so the
p-state ramp (half PE clock until ~3.5us of continuous PE activity)
completes during the DMA-bound phase and never throttles phase B.
Phase B is software-pipelined: the down-proj (d1) matmuls for f-tile f
are issued during f+1's gate/up matmuls so the silu*up scalar/vector
latency hides behind PE streaming. Output is staged to SBUF bf16 and
written back per m-tile on the two hardware DMA rings the moment each
tile's copies land (the write stream is wall-limited, so it starts as
early as possible); the final transfer is half a tile so the
post-compute drain is short.
"""

import numpy as np
import ml_dtypes

import concourse.bacc as bacc
import concourse.mybir as mybir
import concourse.tile as tile
from concourse.bass_utils import run_bass_kernel_spmd

F32 = mybir.dt.float32
BF16 = mybir.dt.bfloat16
BF = ml_dtypes.bfloat16

E = 8
KTOP = 2
H = 1024
FF = 2816
R = 256
KH = H // 128    # 8
KR = R // 128    # 2
KF = FF // 128   # 22
MH = H // 128    # 8

CAP = 512        # per-expert device capacity (capacity factor 1.0)
NCH = 2          # chunks per capacity

_BUILD_CACHE = {}
LAST_RESULT = None


def _build(C, nch):
    """Build the per-core bass program for capacity C split into nch chunks."""
    chunk = C // nch
    AB = 2 * R + C      # per-k block in abuf: [ugc_k | uuc_k | xt_k]
    WB = 3 * R          # per-f block in wbuf: [vg_f | vu_f | udc_f]
    nc = bacc.Bacc()

    abuf = nc.declare_dram_parameter("abuf", [128, KH * AB], BF16, isOutput=False)
    wbuf = nc.declare_dram_parameter("wbuf", [128, KF * WB], BF16, isOutput=False)
    vdp = nc.declare_dram_parameter("vdp", [128, MH * R], BF16, isOutput=False)
    # 32 unused tail columns keep the output row stride a multiple of 64B
    ytp = nc.declare_dram_parameter("ytp", [128, MH * C + 32], BF16, isOutput=True)

    with tile.TileContext(nc) as tc:
        with (
            tc.tile_pool(name="wsb", bufs=1) as wsb,
            tc.tile_pool(name="work", bufs=5) as work,
            tc.tile_pool(name="pmm", bufs=8, space="PSUM") as pmm,
        ):
            ab = wsb.tile([128, KH * AB], BF16, tag="ab")
            wb = wsb.tile([128, KF * WB], BF16, tag="wb")
            vds = wsb.tile([128, MH * R], BF16, tag="vds")
            g1s = wsb.tile([128, KR * C], BF16, tag="g1s")
            u1s = wsb.tile([128, KR * C], BF16, tag="u1s")
            d1s = wsb.tile([128, KR * C], BF16, tag="d1s")
            yt = wsb.tile([128, MH * C], BF16, tag="yt")
            warm = wsb.tile([128, 128], BF16, tag="warm")

            def ugc_k(k, m):
                o = k * AB + m * 128
                return ab[:, o:o + 128]

            def uuc_k(k, m):
                o = k * AB + R + m * 128
                return ab[:, o:o + 128]

            def xt_k(k, c0, w=None):
                o = k * AB + 2 * R + c0
                return ab[:, o:o + (w or chunk)]

            def vg_f(f, k):
                o = f * WB + k * 128
                return wb[:, o:o + 128]

            def vu_f(f, k):
                o = f * WB + R + k * 128
                return wb[:, o:o + 128]

            def udc_f(f, m):
                o = f * WB + 2 * R + m * 128
                return wb[:, o:o + 128]

            # --- PE warm-up: keep the PE continuously busy from
            # sequencer-ready until the first input block lands, so the
            # p-state ramp (half clock for the first ~5us of PE activity)
            # starts as early as possible. Vector memset is the only
            # dependency, so the PE starts right after queue entry; the
            # run of small matmuls bridges the input-DMA latency without
            # delaying the first real matmul by more than one quantum.
            nc.vector.memset(warm[:], 0.0)
            wps = pmm.tile([128, 128], F32, tag="mm", name="wps")
            NWARM = 18
            for i in range(NWARM):
                nc.tensor.matmul(
                    wps[:], warm[:], warm[:],
                    start=(i == 0), stop=(i == NWARM - 1),
                )

            # --- input DMAs: one serial ring (SP), in consumption order.
            # k=0's weights go first as their own small transfer so the
            # first LDWEIGHTS unblocks as early as possible (the scalar
            # ring would serialize them behind its act-table load).
            nc.sync.dma_start(ab[:, 0:2 * R], abuf[:, 0:2 * R])

            # k-blocks stream in pairs: twice the contiguous bytes per
            # partition row (6KB runs) for better DMA-queue throughput on
            # the bandwidth-limited input stream
            def ab_dma(k):
                a0 = k * AB + (2 * R if k == 0 else 0)
                nc.sync.dma_start(
                    ab[:, a0:(k + 2) * AB], abuf[:, a0:(k + 2) * AB]
                )

            def wb_dma(i):
                j = min(i + 4, KF)
                nc.sync.dma_start(
                    wb[:, i * WB:j * WB], wbuf[:, i * WB:j * WB]
                )

            for k in range(0, KH, 2):
                ab_dma(k)
            for i in range(0, KF, 4):
                wb_dma(i)
            nc.sync.dma_start(vds[:], vdp[:])

            # --- phase A: g1T/u1T [R, C] = Ug'/Uu'.T @ xT. k-outer with
            # 4*nch concurrent PSUM accumulators; compute starts on the
            # first k-block and paces the serial input DMA stream.
            # Chunk-split: run the full k-loop for chunk n before chunk
            # n+1, so chunk n's staging copies overlap chunk n+1's matmuls
            # and vector/scalar are free when phase B's silu/mul pipeline
            # primes. (The per-k xt blocks stay resident in SBUF, so the
            # second pass re-reads them for free.)
            psA = [
                pmm.tile([128, chunk], F32, tag="mm", name=f"psA_{n}_{t}_{m}")
                for n in range(nch) for t in range(2) for m in range(KR)
            ]

            def a_mm(n, k):
                for t, wfun in enumerate((ugc_k, uuc_k)):
                    for m in range(KR):
                        nc.tensor.matmul(
                            psA[(n * 2 + t) * KR + m][:],
                            wfun(k, m),
                            xt_k(k, n * chunk),
                            start=(k == 0), stop=(k == KH - 1),
                        )

            for n in range(nch):
                c0 = n * chunk
                for k in range(KH):
                    a_mm(n, k)
                    # chunk 0's k-loop paces the input stream: the PE waits
                    # ~0.4us per k-block for data. Fill those waits with
                    # dummy matmuls so the p-state ramp (half clock until
                    # ~3.5us of continuous PE activity) never resets; the
                    # fillers cost nothing when data is late and at most
                    # one quantum when it isn't.
                    if n == 0 and k < KH - 1:
                        for _ in range(3):
                            nc.tensor.matmul(wps[:], warm[:], warm[:])
                for t, dst in enumerate((g1s, u1s)):
                    for m in range(KR):
                        src = psA[(n * 2 + t) * KR + m][:]
                        d = dst[:, m * C + c0:m * C + c0 + chunk]
                        # last chunk's up-copies go to vector: scalar must be
                        # free at B-start for the first silu (the critical
                        # d1-priming chain)
                        if t == 0 or n == nch - 1:
                            nc.vector.tensor_copy(d, src)
                        else:
                            nc.scalar.activation(
                                d, src, mybir.ActivationFunctionType.Copy
                            )

            # --- phase B: f-loop over KF intermediate tiles. d1 matmuls
            # for f-1 are issued during f's gate/up matmuls (software
            # pipeline) so the silu*up DVE latency never stalls the PE.
            d1p = [
                pmm.tile([128, chunk], F32, tag="mm", name=f"d1p_{n}_{m}")
                for n in range(nch) for m in range(KR)
            ]

            af_hist = {}

            def issue_d1(f, ns):
                afs = af_hist[f]
                for m in range(KR):
                    for n in ns:
                        nc.tensor.matmul(
                            d1p[n * KR + m][:], udc_f(f, m), afs[n][:],
                            start=(f == 0), stop=(f == KF - 1),
                        )

            for f in range(KF):
                gps = [
                    pmm.tile([128, chunk], F32, tag="mm", name=f"gps_{n}_{f}")
                    for n in range(nch)
                ]
                ups = [
                    pmm.tile([128, chunk], F32, tag="mm", name=f"ups_{n}_{f}")
                    for n in range(nch)
                ]
                # n-major: chunk 0's accumulation completes two instructions
                # earlier, buying the scalar silu pipeline slack per f-tile
                for n in range(nch):
                    for k in range(KR):
                        c0 = n * chunk
                        nc.tensor.matmul(
                            gps[n][:], vg_f(f, k),
                            g1s[:, k * C + c0:k * C + c0 + chunk],
                            start=(k == 0), stop=(k == KR - 1),
                        )
                for n in range(nch):
                    for k in range(KR):
                        c0 = n * chunk
                        nc.tensor.matmul(
                            ups[n][:], vu_f(f, k),
                            u1s[:, k * C + c0:k * C + c0 + chunk],
                            start=(k == 0), stop=(k == KR - 1),
                        )
                if f >= 1:
                    issue_d1(f - 1, range(nch))
                    del af_hist[f - 1]
                afs = []
                for n in range(nch):
                    gsil = work.tile([128, chunk], BF16, tag="gsil")
                    nc.scalar.activation(
                        gsil[:], gps[n][:], mybir.ActivationFunctionType.Silu
                    )
                    af = work.tile(
                        [128, chunk], BF16, tag="af", name=f"af_{n}_{f}", bufs=8
                    )
                    nc.vector.tensor_mul(af[:], gsil[:], ups[n][:])
                    afs.append(af)
                    # last f-tile: issue chunk n's d1 matmuls right after
                    # its silu*up lands, instead of waiting for all chunks
                    # - shortens the serial B->C transition bubble.
                    if f == KF - 1:
                        af_hist[f] = afs
                        issue_d1(f, [n])
                af_hist[f] = afs

            # d1 -> bf16 staging; n-outer so phase C's first (m, n=0)
            # accumulation pair unblocks earliest; vector/scalar in parallel.
            for n in range(nch):
                for k in range(KR):
                    c0 = n * chunk
                    src = d1p[n * KR + k][:]
                    d = d1s[:, k * C + c0:k * C + c0 + chunk]
                    if k % 2 == 0:
                        nc.vector.tensor_copy(d, src)
                    else:
                        nc.scalar.activation(
                            d, src, mybir.ActivationFunctionType.Copy
                        )

            # --- phase C: yT [H, C] = Vd.T @ d1T, staged to SBUF bf16,
            # one m-tile DMA issued as soon as that tile's copies land so
            # the write-back streams during phase C's remaining matmuls.
            for m in range(MH):
                ypsl = [
                    pmm.tile([128, chunk], F32, tag="mm", name=f"yps_{n}_{m}")
                    for n in range(nch)
                ]
                # n-outer so chunk n's copy overlaps chunk n+1's matmuls
                for n in range(nch):
                    c0 = n * chunk
                    for k in range(KR):
                        nc.tensor.matmul(
                            ypsl[n][:],
                            vds[:, m * R + k * 128:m * R + (k + 1) * 128],
                            d1s[:, k * C + c0:k * C + c0 + chunk],
                            start=(k == 0), stop=(k == KR - 1),
                        )
                    d = yt[:, m * C + c0:m * C + c0 + chunk]
                    # last m-tile's copies both on vector (faster), so the
                    # final DMA isn't gated on a busy scalar queue
                    if m == MH - 1 or (m * nch + n) % 2 == 0:
                        nc.vector.tensor_copy(d, ypsl[n][:])
                    else:
                        nc.scalar.activation(
                            d, ypsl[n][:], mybir.ActivationFunctionType.Copy
                        )
                # per-m-tile write-back issued the moment the tile's copies
                # land, alternating the two hardware DMA rings (sync /
                # scalar): the ~4us wall-limited output stream starts as
                # early as possible instead of queueing at the end.
                # (gpsimd's software-dynamic path is ~2x slower.)
                if m < MH - 1:
                    eng = nc.sync if m % 2 == 0 else nc.scalar
                    eng.dma_start(
                        ytp[:, m * C:(m + 1) * C], yt[:, m * C:(m + 1) * C]
                    )
                else:
                    # final m-tile: per-chunk transfers split across both
                    # rings so descriptor generation and completion run in
                    # parallel and the very last transfer is half a tile
                    for n in range(nch):
                        c0 = n * chunk
                        eng = nc.scalar if n == 0 else nc.sync
                        eng.dma_start(
                            ytp[:, m * C + c0:m * C + c0 + chunk],
                            yt[:, m * C + c0:m * C + c0 + chunk],
                        )

    nc.finalize()
    return nc


def _pack_k(a, kt):
    """[kt*128, X] -> [128, kt, X] partition-tiled per k."""
    x = a.shape[1]
    return np.ascontiguousarray(a.reshape(kt, 128, x).transpose(1, 0, 2))


def _pack_fmajor(a, kt):
    """[kt*128, ft*128] -> [128, ft, kt*128]: f-major, k tiles adjacent."""
    ft = a.shape[1] // 128
    return np.ascontiguousarray(
        a.reshape(kt, 128, ft, 128).transpose(1, 2, 0, 3).reshape(128, ft, kt * 128)
    )


def _silu(v):
    return v / (1.0 + np.exp(-v))


def kernel(hidden_states, gate_w, Ug, Cg, Vg, Uu, Cu, Vu, Ud, Cd, Vd):
    global LAST_RESULT
    hidden_states = np.asarray(hidden_states, dtype=np.float32)
    gate_w = np.asarray(gate_w, dtype=np.float32)
    b, s, h = hidden_states.shape
    x = hidden_states.reshape(-1, h)
    T = x.shape[0]

    # --- router (host; part of dispatch)
    logits = (x @ gate_w).astype(np.float64)
    lmax = logits.max(axis=-1, keepdims=True)
    p = np.exp(logits - lmax)
    p /= p.sum(axis=-1, keepdims=True)
    i1 = np.argmax(p, axis=-1)
    p1 = p[np.arange(T), i1]
    p_masked = p.copy()
    p_masked[np.arange(T), i1] = -np.inf
    i2 = np.argmax(p_masked, axis=-1)
    p2 = p[np.arange(T), i2]
    w1 = (p1 / (p1 + p2)).astype(np.float32)
    w2 = (p2 / (p1 + p2)).astype(np.float32)

    idx_e = []
    wgt_e = []
    for e in range(E):
        sel1 = np.nonzero(i1 == e)[0]
        sel2 = np.nonzero(i2 == e)[0]
        ids = np.concatenate([sel1, sel2])
        ws = np.concatenate([w1[sel1], w2[sel2]])
        idx_e.append(ids)
        wgt_e.append(ws)

    # Fixed device capacity (capacity factor 1.0); tokens routed past an
    # expert's capacity are handled exactly on the host in the combine.
    C, nch = CAP, NCH

    key = (C, nch)
    if key not in _BUILD_CACHE:
        _BUILD_CACHE[key] = _build(C, nch)
    nc = _BUILD_CACHE[key]

    f32 = np.float32
    in_maps = []
    folded = []
    for e in range(E):
        ids = idx_e[e][:C]
        xT = np.zeros((h, C), f32)
        xT[:, :len(ids)] = x[ids].T
        ugc = (Ug[e] @ Cg).astype(BF)
        uuc = (Uu[e] @ Cu).astype(BF)
        udc = (Ud[e] @ Cd).astype(BF)
        folded.append((ugc, uuc, udc))
        # abuf: per-k contiguous blocks [128, ugc_k | uuc_k | xt_k], flat
        abuf = np.ascontiguousarray(np.concatenate(
            [_pack_k(ugc, KH), _pack_k(uuc, KH), _pack_k(xT.astype(BF), KH)],
            axis=2,
        ).reshape(128, -1))  # [128, KH*AB]
        # wbuf: per-f blocks [vg_f | vu_f | udc_f], flat
        wbuf = np.ascontiguousarray(np.concatenate(
            [
                _pack_fmajor(np.asarray(Vg[e], BF), KR),
                _pack_fmajor(np.asarray(Vu[e], BF), KR),
                _pack_k(udc, KF),
            ],
            axis=2,
        ).reshape(128, -1))  # [128, KF*WB]
        in_maps.append({
            "abuf": abuf,
            "wbuf": wbuf,
            "vdp": np.ascontiguousarray(
                _pack_fmajor(np.asarray(Vd[e], BF), KR).reshape(128, -1)
            ),
        })

    res = run_bass_kernel_spmd(nc, in_maps, list(range(E)))
    LAST_RESULT = res

    out = np.zeros((T, h), f32)
    for e in range(E):
        ids = idx_e[e][:C]
        ytp = np.asarray(res.results[e]["ytp"], dtype=f32)[:, :MH * C]
        yT = ytp.reshape(128, MH, C).transpose(1, 0, 2).reshape(h, C)
        out[ids] += wgt_e[e][:C][:, None] * yT[:, :len(ids)].T
        # capacity overflow: exact host FFN for the few dropped tokens
        over = idx_e[e][C:]
        if len(over):
            ugc, uuc, udc = folded[e]
            xo = x[over]
            g = (xo @ ugc.astype(f32)) @ np.asarray(Vg[e], f32)
            u = (xo @ uuc.astype(f32)) @ np.asarray(Vu[e], f32)
            a = _silu(g) * u
            y = (a @ udc.astype(f32)) @ np.asarray(Vd[e], f32)
            out[over] += wgt_e[e][C:][:, None] * y
    return out.reshape(b, s, h)
